# revision 45
# baseline (speedup 1.0000x reference)
"""Trainium2 Bass kernel for nn_BackflowNet (gnn_message_passing).

Computation per walker b (B=256, N=64, D=3):
    r_ij = x_i - x_j ; feats = [x_i, x_j, r, |r|, |r|^2]  (r folded into W1)
    m_ij = silu(silu(feats @ W1 + b1) @ W2 + b2)          (128-dim messages)
    m_i  = sum_{j != i} m_ij
    out  = tanh(psi([x, m_i])) * bf_scale                 (3-layer MLP psi)

Sharding: pure data parallel over B across 8 cores (32 walkers/core),
params replicated.

Design (ACT-engine-bound, ~305us vs 458us baseline; rel err ~5e-3):
  - The whole message path runs in fp16: full-speed PE matmuls, cheap
    LDWEIGHTS, DVE 2x modes, half the DMA bytes.
  - Walkers are processed in groups of 4. Features live in [128, 2048]
    supertiles (walker q's 8 feature rows at partitions 32q..32q+7,
    rows 8..31 kept zero). L1 (K=8) uses 16-tile (32x32) row+col
    PE-array packing: 16 concurrent quadrant matmuls per 512-pair
    sblock (tile_position=(32q, 32m)), ~10x effective PE throughput.
  - The ACT engine is the bottleneck (~250us of silu work). All silu
    instructions are 2048 columns (4 PSUM banks in one read), and the
    emission is software-pipelined (L2/ACT2 lag L1/ACT1 by one sblock)
    so ACT stays ~95% busy.
  - r1=sqrt(r2) is computed on the DVE (fp16 Quake rsqrt + 1 Newton
    step, 0.26% rel err) so the ACT table never switches to Sqrt; the
    table sequence is just SILU -> TANH.
  - Startup latency: critical weight DMAs are emitted first (the sync
    DMA queue is serial, ~0.6us per dma_start); chunk 0's r build runs
    in 512-col j-pieces so the first L1 starts ~28us in; chunk 1's
    build is emitted mid-stream.
  - The masked j-sum is a fp16 pairwise-add tree on DVE; diagonal
    messages are zeroed by strided memsets beforehand.
  - psi MLP (fp16 weights) runs as a short serial tail with 2048-col
    ACTs and tanh at the very end.
"""

import numpy as np

B, N, D = 256, 64, 3
NCORES = 8
BW = B // NCORES        # walkers per core
MSG_H = 128
HID = 128
NG = BW // 4            # walker groups of 4 per core
NJC = 32                # j-values per chunk
NCHUNK = 2              # chunks per walker (j in [0,32) and [32,64))
GC_COLS = 2048          # pair columns per (group, chunk) supertile


def build_program(bw=BW):
    import concourse.bass as bass
    import concourse.bacc as bacc
    import concourse.tile as tile
    import concourse.mybir as mybir

    F32 = mybir.dt.float32
    F16 = mybir.dt.float16
    F32R = mybir.dt.float32r
    AF = mybir.ActivationFunctionType
    npart = bw * N          # particle rows per core for the psi stage (2048)

    nc = bacc.Bacc("TRN2", target_bir_lowering=False, debug=False)

    xh_h = nc.dram_tensor("xh", [bw, D, N], F16, kind="ExternalInput")
    # w1p replicated at partition bases 0/32/64/96 so each row-tiled L1
    # matmul's weight and fmap start at the same partition index
    w1p_h = nc.dram_tensor("w1p", [128, MSG_H], F16, kind="ExternalInput")
    b1_h = nc.dram_tensor("b1", [MSG_H, 1], F32, kind="ExternalInput")
    w2_h = nc.dram_tensor("w2", [MSG_H, MSG_H], F16, kind="ExternalInput")
    b2_h = nc.dram_tensor("b2", [MSG_H, 1], F32, kind="ExternalInput")
    pw1m_h = nc.dram_tensor("pw1m", [MSG_H, HID], F16, kind="ExternalInput")
    pw1x_h = nc.dram_tensor("pw1x", [D, HID], F16, kind="ExternalInput")
    pb1_h = nc.dram_tensor("pb1", [HID, 1], F32, kind="ExternalInput")
    pw2_h = nc.dram_tensor("pw2", [HID, HID], F16, kind="ExternalInput")
    pb2_h = nc.dram_tensor("pb2", [HID, 1], F32, kind="ExternalInput")
    pw3_h = nc.dram_tensor("pw3", [HID, D], F16, kind="ExternalInput")
    pb3_h = nc.dram_tensor("pb3", [D, 1], F32, kind="ExternalInput")
    sc_h = nc.dram_tensor("sc", [D, 1], F32, kind="ExternalInput")
    out_h = nc.dram_tensor("out", [bw, D, N], F32, kind="ExternalOutput")
    import os
    dbg = os.environ.get("BASS_DBG", "")
    dbg_h = None
    if dbg:
        dbg_h = nc.dram_tensor("dbg", [MSG_H, 8192], F16, kind="ExternalOutput")

    with tile.TileContext(nc) as tc:
        with (
            tc.tile_pool(name="consts", bufs=1) as consts,
            tc.tile_pool(name="build", bufs=1) as build,
            tc.tile_pool(name="rkeep", bufs=1) as rkeep,
            tc.tile_pool(name="feat", bufs=3) as featp,
            tc.tile_pool(name="hpool", bufs=2) as hpool,
            tc.tile_pool(name="mpool", bufs=2) as mpool,
            tc.tile_pool(name="fold", bufs=1) as fold,
            tc.tile_pool(name="tail", bufs=1) as tailp,
            tc.tile_pool(name="ps", bufs=1, space="PSUM") as ps,
        ):
            # ---- critical constants first: the sync DMA queue is serial
            # (~0.6us per dma_start), and the first L1 matmul needs w1p ----
            w1p_t = consts.tile([128, MSG_H], F16)
            nc.sync.dma_start(out=w1p_t, in_=w1p_h.ap())
            b1_t = consts.tile([MSG_H, 1], F32)
            nc.sync.dma_start(out=b1_t, in_=b1_h.ap())
            # m_i accumulator, col = w*64 + i
            stash_t = consts.tile([MSG_H, npart], F16)

            # ---- stacked r build (fp16: 2x DVE mode, half the DMA bytes),
            # layout: partition = 32*d + w over all 32 walkers ----
            r2bf = rkeep.tile([32, 2 * GC_COLS], F16, name="r2bf")
            r1bf = rkeep.tile([32, 2 * GC_COLS], F16, name="r1bf")
            AO = mybir.AluOpType
            U16, I16 = mybir.dt.uint16, mybir.dt.int16
            xjk = [None, None]

            def emit_build_dmas(c):
                XI = build.tile([96, GC_COLS], F16, tag=f"XI{c}",
                                name=f"XI{c}")
                XJc = build.tile([96, NJC], F16, tag=f"XJc{c}",
                                 name=f"XJc{c}")
                for d in range(D):
                    nc.sync.dma_start(
                        out=XI[32 * d:32 * d + 32, :].rearrange(
                            "p (j i) -> p j i", i=N),
                        in_=bass.AP(xh_h, d * N, [[N * D, 32], [0, NJC], [1, N]]),
                    )
                    nc.sync.dma_start(
                        out=XJc[32 * d:32 * d + 32, :],
                        in_=bass.AP(xh_h, d * N + NJC * c,
                                    [[N * D, 32], [1, NJC]]),
                    )
                return XI, XJc

            def emit_build(c, XI, XJc, npiece):
                # chunk 0 is built in 512-col j-pieces so the first sblock's
                # r rows are ready early (subtile deps let L1(s0) start
                # before the rest of the chunk is built)
                xjf = rkeep.tile([96, GC_COLS], F16, name=f"xjf{c}")
                tb = build.tile([32, GC_COLS], F16, tag=f"tb{c}",
                                name=f"tb{c}")
                tc_ = build.tile([32, GC_COLS], F16, tag=f"tc{c}",
                                 name=f"tc{c}")
                t1 = build.tile([32, GC_COLS], F16, tag=f"qt1{c}",
                                name=f"qt1{c}")
                t2 = build.tile([32, GC_COLS], F16, tag=f"qt2{c}",
                                name=f"qt2{c}")
                pw = GC_COLS // npiece
                jw = NJC // npiece
                for p in range(npiece):
                    sl = slice(pw * p, pw * (p + 1))
                    xjbc = XJc[:, jw * p:jw * (p + 1)].unsqueeze(2) \
                        .broadcast_to([96, jw, N])
                    nc.vector.tensor_copy(
                        xjf[:, sl].rearrange("p (j i) -> p j i", i=N), xjbc)
                    nc.vector.tensor_sub(
                        XI[:, sl].rearrange("p (j i) -> p j i", i=N),
                        XI[:, sl].rearrange("p (j i) -> p j i", i=N), xjbc)
                    nc.vector.tensor_mul(XI[:, sl], XI[:, sl], XI[:, sl])
                    nc.sync.dma_start(out=tb[:, sl], in_=XI[32:64, sl])
                    nc.sync.dma_start(out=tc_[:, sl], in_=XI[64:96, sl])
                    nc.vector.tensor_add(tb[:, sl], XI[0:32, sl], tb[:, sl])
                    r2c = r2bf[:, GC_COLS * c:GC_COLS * (c + 1)][:, sl]
                    r1c = r1bf[:, GC_COLS * c:GC_COLS * (c + 1)][:, sl]
                    nc.vector.tensor_add(r2c, tb[:, sl], tc_[:, sl])  # r2
                    # r1 = sqrt(r2+1e-4) on DVE (fp16 Quake rsqrt + 1 Newton
                    # step, ~0.26% rel err): keeps Sqrt off the ACT engine
                    nc.vector.tensor_scalar(r2c, r2c, 1e-4, None, AO.add)
                    nc.vector.tensor_scalar(
                        t1[:, sl].bitcast(U16), r2c.bitcast(U16),
                        1, None, AO.logical_shift_right)
                    nc.vector.tensor_scalar(
                        t1[:, sl].bitcast(I16), t1[:, sl].bitcast(I16),
                        0x59b8, -1, AO.subtract, AO.mult)
                    nc.vector.tensor_mul(t2[:, sl], t1[:, sl], t1[:, sl])
                    nc.vector.tensor_mul(t2[:, sl], t2[:, sl], r2c)
                    nc.vector.tensor_scalar(t2[:, sl], t2[:, sl], -0.5, 1.5,
                                            AO.mult, AO.add)
                    nc.vector.tensor_mul(t2[:, sl], t1[:, sl], t2[:, sl])
                    nc.vector.tensor_mul(r1c, r2c, t2[:, sl])
                xjk[c] = xjf

            XI0, XJc0 = emit_build_dmas(0)
            emit_build(0, XI0, XJc0, 4)
            # w2/b2 are first needed by L2 of sblock 0 (~31us in) — emitted
            # after the startup-critical build DMAs
            w2_t = consts.tile([MSG_H, MSG_H], F16)
            nc.sync.dma_start(out=w2_t, in_=w2_h.ap())
            b2_t = consts.tile([MSG_H, 1], F32)
            nc.sync.dma_start(out=b2_t, in_=b2_h.ap())
            r2k = [r2bf[:, GC_COLS * c:GC_COLS * (c + 1)] for c in range(NCHUNK)]
            r1k = [r1bf[:, GC_COLS * c:GC_COLS * (c + 1)] for c in range(NCHUNK)]

            # ---- main stream over 16 (group, chunk) units, 4 sblocks each
            # unit u: gc = u, sblocks pipelined; ACT2/L2 lag ACT1/L1 by one
            # sblock so the ACT engine never drains.
            gcs = [(g, c) for c in range(NCHUNK) for g in range(NG)]

            # three manually-rotated feature buffers, zeroed once so rows
            # 8..31 of each quadrant stay exactly 0.0 (the K=32 L1 matmul
            # contracts them against zero weight rows)
            F_bufs = []
            for fi in range(3):
                Fb = featp.tile([128, GC_COLS], F16, name=f"Fbuf{fi}")
                nc.vector.memset(Fb, 0.0)
                F_bufs.append(Fb)

            def emit_feat_dmas(g, c, idx):
                # NOTE: DMA out-APs only honor a single partition dim, so one
                # DMA per feature row (partition stride 32 across walkers).
                # For the very first group the xj/r scatters are split per
                # 512-col piece so L1(s0) isn't gated on the whole chunk.
                F = F_bufs[idx % 3]
                for d in range(D):
                    # xi row d: from HBM, broadcast over j
                    nc.sync.dma_start(
                        out=F[d:128:32, :].rearrange("q (j i) -> q j i", i=N),
                        in_=bass.AP(xh_h, 4 * g * N * D + d * N,
                                    [[N * D, 4], [0, NJC], [1, N]]),
                    )
                pieces = 4 if (g, c) == (0, 0) else 1
                pw = GC_COLS // pieces
                for p in range(pieces):
                    sl = slice(pw * p, pw * (p + 1))
                    for d in range(D):
                        # xj row 3+d: from stacked fp16 broadcast tile
                        nc.sync.dma_start(
                            out=F[3 + d:128:32, sl],
                            in_=xjk[c][32 * d + 4 * g:32 * d + 4 * g + 4, sl],
                        )
                    # r2 row 6, r1 row 7
                    nc.sync.dma_start(out=F[6:128:32, sl],
                                      in_=r2k[c][4 * g:4 * g + 4, sl])
                    nc.sync.dma_start(out=F[7:128:32, sl],
                                      in_=r1k[c][4 * g:4 * g + 4, sl])
                return F

            state = {}  # pipelined lag-1 state: (F, h, psA) of previous sblock

            def emit_l1(F, s):
                # 16-tile (32K x 32M) row+col PE-array packing: all 16
                # matmuls run concurrently (distinct row/col groups)
                psA = ps.tile([MSG_H, GC_COLS], F32, tag="A")
                for q in range(4):
                    for mq in range(4):
                        nc.tensor.matmul(
                            psA[32 * mq:32 * (mq + 1), 512 * q:512 * (q + 1)],
                            w1p_t[32 * q:32 * q + 32, 32 * mq:32 * (mq + 1)],
                            F[32 * q:32 * q + 32, 512 * s:512 * (s + 1)],
                            start=True, stop=True,
                            tile_position=(32 * q, 32 * mq),
                        )
                return psA

            def emit_l2(h, s, m):
                psB = ps.tile([MSG_H, GC_COLS], F32, tag="B")
                for k in range(4):
                    nc.tensor.matmul(
                        psB[:, 512 * k:512 * (k + 1)],
                        w2_t,
                        h[:, 2048 * s + 512 * k:2048 * s + 512 * (k + 1)],
                        start=True, stop=True,
                    )
                nc.scalar.activation(m[:, 2048 * s:2048 * (s + 1)], psB,
                                     AF.Silu, bias=b2_t, scale=1.0)

            def emit_fold(g, c, m):
                # zero self-messages: col = 512q + 2056s + 65jj + 32c
                for s in range(4):
                    v = m[:, 2048 * s:2048 * (s + 1)].rearrange(
                        "p (q b) -> p q b", q=4)
                    nc.vector.memset(v[:, :, 32 * c + 8 * s::65], 0.0)
                # fold 32 j -> per-i sums; all fp16 on DVE
                v3 = fold.tile([MSG_H, 1024], F16, tag="v3")
                for s in range(4):
                    ms = m[:, 2048 * s:2048 * (s + 1)].rearrange(
                        "p (q b) -> p q b", q=4)
                    v1 = fold.tile([MSG_H, 1024], F16, tag="v1")
                    v1q = v1.rearrange("p (q b) -> p q b", q=4)
                    nc.vector.tensor_add(v1q, ms[:, :, 0:256], ms[:, :, 256:512])
                    v2 = fold.tile([MSG_H, 512], F16, tag="v2")
                    v2q = v2.rearrange("p (q b) -> p q b", q=4)
                    nc.vector.tensor_add(v2q, v1q[:, :, 0:128], v1q[:, :, 128:256])
                    nc.vector.tensor_add(
                        v3[:, 256 * s:256 * (s + 1)].rearrange(
                            "p (q b) -> p q b", q=4),
                        v2q[:, :, 0:64], v2q[:, :, 64:128])
                ta = fold.tile([MSG_H, 256], F16, tag="ta")
                nc.vector.tensor_add(ta, v3[:, 0:256], v3[:, 256:512])
                tb2 = fold.tile([MSG_H, 256], F16, tag="tb2")
                nc.vector.tensor_add(tb2, v3[:, 512:768], v3[:, 768:1024])
                sl = stash_t[:, 256 * g:256 * (g + 1)]
                if c == 0:
                    nc.vector.tensor_add(sl, ta, tb2)
                else:
                    tc2 = fold.tile([MSG_H, 256], F16, tag="tc2")
                    nc.vector.tensor_add(tc2, ta, tb2)
                    nc.vector.tensor_add(sl, sl, tc2)
                if dbg == "m0" and (g, c) == (0, 0):
                    nc.sync.dma_start(
                        out=bass.AP(dbg_h, 0, [[8192, MSG_H], [1, 8192]]),
                        in_=m)

            prev = None  # (g, c, h, m, s) of the lagging sblock stream
            for idx, (g, c) in enumerate(gcs):
                if idx == 4:
                    # chunk-1 r build: emitted mid-stream so its DMAs/DVE
                    # ops don't crowd the startup-critical queues
                    XI1, XJc1 = emit_build_dmas(1)
                    emit_build(1, XI1, XJc1, 2)
                F = emit_feat_dmas(g, c, idx)
                h = hpool.tile([MSG_H, 4 * GC_COLS], F16, tag="h",
                               name=f"h_{g}_{c}")
                m = mpool.tile([MSG_H, 4 * GC_COLS], F16, tag="m",
                               name=f"m_{g}_{c}")
                for s in range(4):
                    # Steady state: ACT1 of this sblock before ACT2 of the
                    # lagging one (PE gets the ACT2 window to run L1+L2).
                    # At the group boundary (s==0) the lagging ACT2 goes
                    # FIRST so ACT1(g, s0) doesn't eat the fresh PE-sem
                    # propagation latency.
                    pl2 = prev
                    if s == 0 and pl2 is not None:
                        pg, pc, ph, pm, psb = pl2
                        emit_l2(ph, psb, pm)
                        if psb == 3:
                            emit_fold(pg, pc, pm)
                        pl2 = None
                    psA = emit_l1(F, s)
                    nc.scalar.activation(h[:, 2048 * s:2048 * (s + 1)], psA,
                                         AF.Silu, bias=b1_t, scale=1.0)
                    if pl2 is not None:
                        pg, pc, ph, pm, psb = pl2
                        emit_l2(ph, psb, pm)
                        if psb == 3:
                            emit_fold(pg, pc, pm)
                    prev = (g, c, h, m, s)
                if dbg == "h0" and (g, c) == (0, 0):
                    nc.sync.dma_start(out=bass.AP(dbg_h, 0, [[8192, MSG_H], [1, 8192]]), in_=h)
                if dbg == "F0" and (g, c) == (0, 0):
                    nc.sync.dma_start(out=bass.AP(dbg_h, 0, [[8192, MSG_H], [1, 2048]]), in_=F)
            # drain the lagging sblock
            pg, pc, ph, pm, psb = prev
            emit_l2(ph, psb, pm)
            emit_fold(pg, pc, pm)
            if dbg == "m0":
                pass  # handled inside loop for gc (0,0) via m dump below
            if dbg == "stash":
                nc.sync.dma_start(
                    out=bass.AP(dbg_h, 0, [[8192, MSG_H], [1, 2048]]),
                    in_=stash_t)

            # ---- psi tail constants (emitted late so their DMAs don't
            # crowd the startup-critical sync queue) ----
            pw1m_t = consts.tile([MSG_H, HID], F16)
            nc.sync.dma_start(out=pw1m_t, in_=pw1m_h.ap())
            pw1x_t = consts.tile([D, HID], F16)
            nc.sync.dma_start(out=pw1x_t, in_=pw1x_h.ap())
            pw2_t = consts.tile([HID, HID], F16)
            nc.sync.dma_start(out=pw2_t, in_=pw2_h.ap())
            pw3_t = consts.tile([HID, D], F16)
            nc.sync.dma_start(out=pw3_t, in_=pw3_h.ap())
            pb1_t = consts.tile([HID, 1], F32)
            nc.sync.dma_start(out=pb1_t, in_=pb1_h.ap())
            pb2_t = consts.tile([HID, 1], F32)
            nc.sync.dma_start(out=pb2_t, in_=pb2_h.ap())
            pb3_t = consts.tile([D, 1], F32)
            nc.sync.dma_start(out=pb3_t, in_=pb3_h.ap())
            sc_t = consts.tile([D, 1], F32)
            nc.sync.dma_start(out=sc_t, in_=sc_h.ap())
            # xT: [3, bw*64] fp16, col = w*64 + i (psi input / output layout)
            xT_t = consts.tile([D, npart], F16)
            nc.sync.dma_start(
                out=xT_t,
                in_=bass.AP(xh_h, 0, [[N, D], [N * D, bw], [1, N]]),
            )

            # ---- psi MLP tail over all particle rows ----
            u1 = tailp.tile([HID, npart], F16)
            u2 = tailp.tile([HID, npart], F16)
            dxs = tailp.tile([D, npart], F32)
            psA = ps.tile([HID, npart], F32, tag="A")
            for s in range(4):
                sl = slice(512 * s, 512 * (s + 1))
                nc.tensor.matmul(psA[:, sl], pw1m_t, stash_t[:, sl],
                                 start=True, stop=False)
                nc.tensor.matmul(psA[:, sl], pw1x_t, xT_t[:, sl],
                                 start=False, stop=True)
            nc.scalar.activation(u1, psA, AF.Silu, bias=pb1_t, scale=1.0)
            psB = ps.tile([HID, npart], F32, tag="B")
            for s in range(4):
                sl = slice(512 * s, 512 * (s + 1))
                nc.tensor.matmul(psB[:, sl], pw2_t, u1[:, sl],
                                 start=True, stop=True)
            nc.scalar.activation(u2, psB, AF.Silu, bias=pb2_t, scale=1.0)
            psD = ps.tile([D, npart], F32, tag="A")
            for s in range(4):
                sl = slice(512 * s, 512 * (s + 1))
                nc.tensor.matmul(psD[:, sl], pw3_t, u2[:, sl],
                                 start=True, stop=True)
            nc.scalar.activation(dxs, psD, AF.Tanh, bias=pb3_t, scale=1.0)
            nc.vector.tensor_scalar_mul(dxs, dxs, sc_t)
            nc.sync.dma_start(
                out=bass.AP(out_h, 0, [[N, D], [N * D, bw], [1, N]]),
                in_=dxs.rearrange("p (w i) -> p w i", i=N),
            )

    nc.compile()
    return nc


def host_inputs(x, phi_w1, phi_b1, phi_w2, phi_b2,
                psi_w1, psi_b1, psi_w2, psi_b2, psi_w3, psi_b3, bf_scale,
                bw=BW, ncores=NCORES):
    """Per-core in_maps from the full problem inputs."""
    F16 = np.float16
    x = np.asarray(x, np.float32)
    w1 = np.asarray(phi_w1, np.float64)
    w1p = np.concatenate([
        w1[0:3] + w1[6:9],      # xi rows (r folded in)
        w1[3:6] - w1[6:9],      # xj rows
        w1[10:11],              # r2
        w1[9:10],               # r1
    ], axis=0)
    sc = np.maximum(np.float32(bf_scale), 0.0)
    w1p4 = np.zeros((128, MSG_H), np.float64)
    for q in range(4):
        w1p4[32 * q:32 * q + 8] = w1p
    const = {
        "w1p": w1p4.astype(F16),
        "b1": np.asarray(phi_b1, np.float32).reshape(MSG_H, 1),
        "w2": np.asarray(phi_w2, F16),
        "b2": np.asarray(phi_b2, np.float32).reshape(MSG_H, 1),
        "pw1x": np.ascontiguousarray(np.asarray(psi_w1, F16)[0:3]),
        "pw1m": np.ascontiguousarray(np.asarray(psi_w1, F16)[3:]),
        "pb1": np.asarray(psi_b1, np.float32).reshape(HID, 1),
        "pw2": np.asarray(psi_w2, F16),
        "pb2": np.asarray(psi_b2, np.float32).reshape(HID, 1),
        "pw3": np.asarray(psi_w3, F16),
        "pb3": np.asarray(psi_b3, np.float32).reshape(D, 1),
        "sc": np.full((D, 1), sc, np.float32),
    }
    in_maps = []
    for core in range(ncores):
        xs = np.ascontiguousarray(
            x[core * bw:(core + 1) * bw].transpose(0, 2, 1))
        in_maps.append({"xh": xs.astype(F16), **const})
    return in_maps


_cached_nc = None
LAST_EXEC_NS = None
LAST_PROFILE_JSON = None
LAST_TRACE_PATH = None


def kernel(x, spin, phi_w1, phi_b1, phi_w2, phi_b2,
           psi_w1, psi_b1, psi_w2, psi_b2, psi_w3, psi_b3, bf_scale):
    global _cached_nc
    from concourse.bass_utils import run_bass_kernel_spmd

    if _cached_nc is None:
        _cached_nc = build_program()
    in_maps = host_inputs(x, phi_w1, phi_b1, phi_w2, phi_b2,
                          psi_w1, psi_b1, psi_w2, psi_b2, psi_w3, psi_b3,
                          bf_scale)
    import os
    trace = bool(os.environ.get("BASS_TRACE"))
    res = run_bass_kernel_spmd(_cached_nc, in_maps, core_ids=list(range(NCORES)),
                               trace=trace)
    global LAST_EXEC_NS, LAST_PROFILE_JSON, LAST_TRACE_PATH
    if res.exec_time_ns is not None:
        LAST_EXEC_NS = res.exec_time_ns
    if res.profile_json is not None:
        LAST_PROFILE_JSON = res.profile_json
    if res.instructions_and_trace is not None:
        LAST_TRACE_PATH = res.instructions_and_trace[1]
    out = np.concatenate(
        [r["out"].transpose(0, 2, 1) for r in res.results], axis=0)
    return out.astype(np.float32)
